# revision 8
# baseline (speedup 1.0000x reference)
"""Trainium2 Bass kernel for CAttentionBlock (windowed multi-head attention x4 + LN).

Computation per batch image (one NeuronCore each, pure data parallel over B=8):
  window-partition (2x2 windows, N=4 tokens, C=256, NH=8 heads, d=32)
  x1 = LN1(rw + attn(rw, gw, gw))
  x2 = LN2(gw + attn(gw, bw, bw))
  x3 = LN3(bw + attn(bw, iw, iw))
  x4 = LN4(iw + attn(iw, gw, gw))
  out = concat([x1, x2, x3, x4], -1)  -> [64, 64, 1024]

Layout: windows on SBUF partitions (128 windows/tile, 8 tiles/core), tokens x
channels on the free dim ([128, 4*256]).  The tiny 4x4 attention is computed
with broadcast access patterns on the vector engine; scalar engine does exp and
LN statistics (via accum_out); gpsimd does residual/bias adds; no matmuls.
"""

import sys

for _p in ("/opt/trn_rl_repo",):
    if _p not in sys.path:
        sys.path.insert(0, _p)

import numpy as np

import bass_rust
import concourse.bass as bass
import concourse.tile as tile
from concourse import mybir
from concourse.bass_utils import run_bass_kernel_spmd

F32 = mybir.dt.float32
ALU = mybir.AluOpType
ACT = mybir.ActivationFunctionType

B, H, W, C = 8, 64, 64, 256
WS = 2
NH = 8
D = C // NH            # 32
NTOK = WS * WS         # 4
NW = (H // WS) * (W // WS)   # 1024 windows per image
NWI = H // WS          # 32 window rows
P = 128                # partitions / windows per tile
NTILES = NW // P       # 8
EPS = 1e-5
INV_SQRT_D = 1.0 / float(np.sqrt(D))

# (q_tensor_idx, kv_tensor_idx) per attention; tensors ordered r,g,b,ir
ATTNS = [(0, 1), (1, 2), (2, 3), (3, 1)]


def _ap(ref, offset_delta, dims):
    """Build an AP on ref's tensor at ref.offset + delta with explicit
    [step, count] dims (elements)."""
    return bass_rust.AP(ref.tensor, ref.offset + offset_delta, [list(d) for d in dims])


def build_kernel():
    nc = bass.Bass("TRN2", target_bir_lowering=False, debug=False)

    ins = {
        name: nc.dram_tensor(name, [H, W, C], F32, kind="ExternalInput")
        for name in ("r", "g", "b", "ir")
    }
    ln_params = []
    for a in range(4):
        wv = nc.dram_tensor(f"ln{a + 1}_w", [C], F32, kind="ExternalInput")
        bv = nc.dram_tensor(f"ln{a + 1}_b", [C], F32, kind="ExternalInput")
        ln_params.append((wv, bv))
    out = nc.dram_tensor("out", [H, W, 4 * C], F32, kind="ExternalOutput")

    in_aps = [ins[n].ap() for n in ("r", "g", "b", "ir")]
    out_ap = out.ap()

    with tile.TileContext(nc) as tc:
        with (
            tc.tile_pool(name="const", bufs=1) as pconst,
            tc.tile_pool(name="pin", bufs=2) as pin,
            tc.tile_pool(name="pbig", bufs=2) as pbig,
            tc.tile_pool(name="pmed", bufs=2) as pmed,
            tc.tile_pool(name="psmall", bufs=2) as psmall,
        ):
            # LN weight/bias replicated to all 128 partitions
            wreps, breps = [], []
            for a, (wv, bv) in enumerate(ln_params):
                wt = pconst.tile([P, C], F32, tag=f"wrep{a}")
                bt = pconst.tile([P, C], F32, tag=f"brep{a}")
                nc.sync.dma_start(
                    out=wt[:], in_=_ap(wv.ap(), 0, [[0, P], [1, C]])
                )
                nc.sync.dma_start(
                    out=bt[:], in_=_ap(bv.ap(), 0, [[0, P], [1, C]])
                )
                wreps.append(wt)
                breps.append(bt)

            for t in range(NTILES):
                # ---- load the 4 input tiles: [128 windows, 4 tok * 256 ch]
                tiles = []
                for xi, name in enumerate(("r", "g", "b", "ir")):
                    tx = pin.tile([P, NTOK * C], F32, tag=f"in{name}")
                    txr = tx[:]
                    for qh in range(2):
                        # one DMA covers all 128 windows' (qh, qw, c) half:
                        # DRAM run (j, qw, c) is 16K elems contiguous; the
                        # SBUF side stays canonical [128, F] so Tile's
                        # dependency tracking sees the true partition range.
                        src = _ap(
                            in_aps[xi],
                            (8 * t + qh) * W * C,
                            [[2 * W * C, 4], [2 * C, NWI], [1, 2 * C]],
                        )
                        dst = _ap(txr, qh * 2 * C, [txr.ap[0], [1, 2 * C]])
                        nc.sync.dma_start(out=dst, in_=src)
                    tiles.append(tx)

                for a, (qi, ki) in enumerate(ATTNS):
                    qt = tiles[qi]
                    kt = tiles[ki]
                    qr = qt[:]
                    kr = kt[:]

                    # ---- scores: prod[w,(q,k,c)] = Q[w,q,c] * K[w,k,c]
                    prod = pbig.tile([P, 16 * C], F32, tag="prod")
                    pr = prod[:]
                    nc.vector.tensor_tensor(
                        out=_ap(pr, 0, [pr.ap[0], [4 * C, 4], [C, 4], [1, C]]),
                        in0=_ap(qr, 0, [qr.ap[0], [C, 4], [0, 4], [1, C]]),
                        in1=_ap(kr, 0, [kr.ap[0], [0, 4], [C, 4], [1, C]]),
                        op=ALU.mult,
                    )
                    # s[w,(q,k,h)] = sum_d prod
                    s = psmall.tile([P, 128], F32, tag="s")
                    sr = s[:]
                    nc.vector.reduce_sum(
                        out=_ap(sr, 0, [sr.ap[0], [32, 4], [8, 4], [1, 8]]),
                        in_=_ap(
                            pr, 0, [pr.ap[0], [4 * C, 4], [C, 4], [D, 8], [1, D]]
                        ),
                        axis=mybir.AxisListType.X,
                    )
                    # e = exp(s / sqrt(d))   (no max-subtraction: scores ~ N(0,1))
                    e = psmall.tile([P, 128], F32, tag="e")
                    nc.scalar.activation(
                        out=e[:], in_=s[:], func=ACT.Exp, bias=0.0, scale=INV_SQRT_D
                    )
                    # Z[w,(q,h)] = sum_k e ; rinv = 1/Z
                    z = psmall.tile([P, 32], F32, tag="z")
                    er = e[:]
                    nc.vector.reduce_sum(
                        out=z[:],
                        in_=_ap(er, 0, [er.ap[0], [32, 4], [1, 8], [8, 4]]),
                        axis=mybir.AxisListType.X,
                    )
                    rz = psmall.tile([P, 32], F32, tag="rz")
                    nc.vector.reciprocal(out=rz[:], in_=z[:])
                    # pn[w,(q,k,h)] = e * rinv (broadcast over k)
                    pn = psmall.tile([P, 128], F32, tag="pn")
                    rzr = rz[:]
                    nc.vector.tensor_tensor(
                        out=pn[:],
                        in0=e[:],
                        in1=_ap(rzr, 0, [rzr.ap[0], [8, 4], [0, 4], [1, 8]]),
                        op=ALU.mult,
                    )
                    # ---- AV: prodv[w,(q,k,c)] = pn[w,q,k,h(c)] * KV[w,k,c]
                    prodv = pbig.tile([P, 16 * C], F32, tag="prodv")
                    pvr = prodv[:]
                    pnr = pn[:]
                    nc.vector.tensor_tensor(
                        out=_ap(
                            pvr, 0, [pvr.ap[0], [4 * C, 4], [C, 4], [D, 8], [1, D]]
                        ),
                        in0=_ap(pnr, 0, [pnr.ap[0], [32, 4], [8, 4], [1, 8], [0, D]]),
                        in1=_ap(kr, 0, [kr.ap[0], [0, 4], [C, 4], [D, 8], [1, D]]),
                        op=ALU.mult,
                    )
                    # k-sum: av[w,(q,c)] = sum_k prodv
                    t01 = pmed.tile([P, NTOK * C], F32, tag="t01")
                    t23 = pmed.tile([P, NTOK * C], F32, tag="t23")
                    av = pmed.tile([P, NTOK * C], F32, tag="av")
                    qk_dims = [pvr.ap[0], [4 * C, 4], [1, C]]
                    nc.vector.tensor_tensor(
                        out=t01[:],
                        in0=_ap(pvr, 0 * C, qk_dims),
                        in1=_ap(pvr, 1 * C, qk_dims),
                        op=ALU.add,
                    )
                    nc.vector.tensor_tensor(
                        out=t23[:],
                        in0=_ap(pvr, 2 * C, qk_dims),
                        in1=_ap(pvr, 3 * C, qk_dims),
                        op=ALU.add,
                    )
                    nc.vector.tensor_tensor(
                        out=av[:], in0=t01[:], in1=t23[:], op=ALU.add
                    )
                    # ---- residual (gpsimd)
                    xres = pmed.tile([P, NTOK * C], F32, tag="xres")
                    nc.gpsimd.tensor_tensor(
                        out=xres[:], in0=av[:], in1=qr, op=ALU.add
                    )
                    # ---- LN stats (scalar engine accumulators)
                    msum = psmall.tile([P, NTOK], F32, tag="msum")
                    ssq = psmall.tile([P, NTOK], F32, tag="ssq")
                    scr = pmed.tile([P, NTOK * C], F32, tag="scr")
                    scr2 = pmed.tile([P, NTOK * C], F32, tag="scr2")
                    for q in range(NTOK):
                        qs = slice(q * C, (q + 1) * C)
                        nc.scalar.activation(
                            out=scr[:, qs],
                            in_=xres[:, qs],
                            func=ACT.Copy,
                            bias=0.0,
                            scale=1.0,
                            accum_out=msum[:, q : q + 1],
                        )
                        nc.scalar.activation(
                            out=scr2[:, qs],
                            in_=xres[:, qs],
                            func=ACT.Square,
                            bias=0.0,
                            scale=1.0,
                            accum_out=ssq[:, q : q + 1],
                        )
                    mu = psmall.tile([P, NTOK], F32, tag="mu")
                    nc.vector.tensor_scalar(
                        out=mu[:], in0=msum[:], scalar1=1.0 / C, scalar2=None,
                        op0=ALU.mult,
                    )
                    ex2 = psmall.tile([P, NTOK], F32, tag="ex2")
                    nc.vector.tensor_scalar(
                        out=ex2[:], in0=ssq[:], scalar1=1.0 / C, scalar2=None,
                        op0=ALU.mult,
                    )
                    var = psmall.tile([P, NTOK], F32, tag="var")
                    nc.vector.tensor_tensor(
                        out=var[:], in0=mu[:], in1=mu[:], op=ALU.mult
                    )
                    nc.vector.tensor_tensor(
                        out=var[:], in0=ex2[:], in1=var[:], op=ALU.subtract
                    )
                    vpe = psmall.tile([P, NTOK], F32, tag="vpe")
                    nc.vector.tensor_scalar(
                        out=vpe[:], in0=var[:], scalar1=EPS, scalar2=None, op0=ALU.add
                    )
                    rinv = psmall.tile([P, NTOK], F32, tag="rinv")
                    nc.vector.reciprocal(out=rinv[:], in_=vpe[:])
                    rs = psmall.tile([P, NTOK], F32, tag="rs")
                    nc.scalar.activation(
                        out=rs[:], in_=rinv[:], func=ACT.Sqrt, bias=0.0, scale=1.0
                    )
                    nmusr = psmall.tile([P, NTOK], F32, tag="nmusr")
                    nc.vector.tensor_tensor(
                        out=nmusr[:], in0=mu[:], in1=rs[:], op=ALU.mult
                    )
                    nc.vector.tensor_scalar(
                        out=nmusr[:], in0=nmusr[:], scalar1=-1.0, scalar2=None,
                        op0=ALU.mult,
                    )
                    # ---- normalize + affine
                    xn = pmed.tile([P, NTOK * C], F32, tag="xn")
                    for q in range(NTOK):
                        qs = slice(q * C, (q + 1) * C)
                        nc.scalar.activation(
                            out=xn[:, qs],
                            in_=xres[:, qs],
                            func=ACT.Identity,
                            bias=nmusr[:, q : q + 1],
                            scale=rs[:, q : q + 1],
                        )
                    y = pmed.tile([P, NTOK * C], F32, tag="y")
                    xnr = xn[:]
                    wr = wreps[a][:]
                    yref = y[:]
                    nc.vector.tensor_tensor(
                        out=_ap(yref, 0, [yref.ap[0], [C, 4], [1, C]]),
                        in0=_ap(xnr, 0, [xnr.ap[0], [C, 4], [1, C]]),
                        in1=_ap(wr, 0, [wr.ap[0], [0, 4], [1, C]]),
                        op=ALU.mult,
                    )
                    xout = pmed.tile([P, NTOK * C], F32, tag="xout")
                    br_ = breps[a][:]
                    yr = y[:]
                    nc.gpsimd.tensor_tensor(
                        out=_ap(xout[:], 0, [xout[:].ap[0], [C, 4], [1, C]]),
                        in0=_ap(yr, 0, [yr.ap[0], [C, 4], [1, C]]),
                        in1=_ap(br_, 0, [br_.ap[0], [0, 4], [1, C]]),
                        op=ALU.add,
                    )
                    # ---- store: out[2i+qh, 2j+qw, a*256:(a+1)*256]
                    xoutr = xout[:]
                    for qh in range(2):
                        for qw in range(2):
                            dst = _ap(
                                out_ap,
                                (8 * t + qh) * W * 4 * C + qw * 4 * C + a * C,
                                [[2 * W * 4 * C, 4], [2 * 4 * C, NWI], [1, C]],
                            )
                            src = _ap(
                                xoutr, qh * 2 * C + qw * C, [xoutr.ap[0], [1, C]]
                            )
                            nc.sync.dma_start(out=dst, in_=src)
    return nc


def _split_multi_waits(nc):
    """Walrus on this toolchain accepts at most one embedded sync-wait per
    instruction; Tile attaches several.  Hoist all but the last wait of each
    instruction into standalone InstEventSemaphore waits on the same engine,
    inserted immediately before it (same blocking semantics)."""
    wid = 0
    for fn in nc.m.functions:
        for blk in fn.blocks:
            new_list = []
            changed = False
            for inst in blk.instructions:
                si = inst.sync_info
                if si is not None:
                    waits = list(si.on_wait)
                    if len(waits) > 1:
                        for w in waits[:-1]:
                            ev = mybir.InstEventSemaphore(
                                name=f"WSPLIT-{wid}", ins=[], outs=[]
                            )
                            wid += 1
                            ev.engine = inst.engine
                            ev.sync_info = bass_rust.SyncInfo(
                                on_wait=[w], on_update=[]
                            )
                            new_list.append(ev)
                        inst.sync_info = bass_rust.SyncInfo(
                            on_wait=[waits[-1]], on_update=list(si.on_update)
                        )
                        changed = True
                new_list.append(inst)
            if changed:
                blk.instructions = new_list


_NC_CACHE = None


def _get_nc():
    global _NC_CACHE
    if _NC_CACHE is None:
        nc = build_kernel()
        _split_multi_waits(nc)
        _NC_CACHE = nc
    return _NC_CACHE


def kernel(**inputs) -> np.ndarray:
    nc = _get_nc()
    param_names = [f"ln{a + 1}_{s}" for a in range(4) for s in ("w", "b")]
    in_maps = []
    for ci in range(B):
        m = {
            name: np.ascontiguousarray(np.asarray(inputs[name])[ci], dtype=np.float32)
            for name in ("r", "g", "b", "ir")
        }
        for pnm in param_names:
            m[pnm] = np.ascontiguousarray(np.asarray(inputs[pnm]), dtype=np.float32)
        in_maps.append(m)
    res = run_bass_kernel_spmd(nc, in_maps, list(range(B)))
    return np.stack([res.results[ci]["out"] for ci in range(B)], axis=0)


if __name__ == "__main__":
    rng = np.random.default_rng(0)
    demo = {n: rng.standard_normal((B, H, W, C), dtype=np.float32) for n in ("r", "g", "b", "ir")}
    for a in range(4):
        demo[f"ln{a + 1}_w"] = rng.standard_normal(C).astype(np.float32)
        demo[f"ln{a + 1}_b"] = rng.standard_normal(C).astype(np.float32)
    o = kernel(**demo)
    print(o.shape, o.dtype)
